# revision 2
# baseline (speedup 1.0000x reference)
"""TRN2 Bass kernel for GCNConv-diag: out = A @ (input * diag(W)).

Strategy (8 NeuronCores, SPMD):
  - Shard A row-wise: core i owns rows [i*1024, (i+1)*1024).
  - Replicate the feature matrix `input` (matmul moving operand) and W.
  - W commutes into the features: fold W into x on the host
    (x = input * W), so the device matmul is plain A @ x.
  - Mean-subtraction + fp8 DoubleRow: A = 0.5*J + B with B = A - 0.5
    (A is uniform[0,1), so B is symmetric in [-0.5, 0.5)).  Then
      A @ x = 0.5 * colsum(x) (rank-1, exact, host-computed)  +  B @ x.
    B @ x runs on the PE in float8e4 (e4m3) with perf_mode=DoubleRow:
    2 fp8 weights per PE cell -> one matmul instruction contracts 2
    k-tiles (K=256), ~2x the fp16 streaming rate.  Removing the mean
    first halves the fp8 quantization error (CPU-verified 1.83e-2
    Frobenius rel err vs 3.7e-2 without; gate is 2e-2).  The exact
    rank-1 mean term is added during the PSUM drain (one DVE add).
  - Optional accuracy dial: the last NF16 k-tiles run in fp16 (error
    scales as sqrt(NF8/64)).  NF8=64 -> pure fp8.
  - Host pre-arranges shards k-major per output tile so every DMA is
    large and fully coalesced (same layout as the fp16 baseline).

Per-core work: out[1024,512] = B_shard[1024,8192] @ x[8192,512]:
8 m-tiles x 32 DoubleRow matmuls ([128,2,128]^T x [128,2,512] -> psum
[128,512], 32-deep accumulation).  PE roofline ~62-76us/core vs 109us
for fp16; HBM ~14.3MB/core @ ~358GB/s = 40us -> PE-bound.
"""

import numpy as np
import ml_dtypes

import concourse.bass as bass
import concourse.tile as tile
from concourse import bacc, mybir
from concourse.bass_utils import run_bass_kernel_spmd

N = 8192  # graph nodes (A is [N, N])
D = 512  # feature dim
NCORES = 8
RPC = N // NCORES  # 1024 rows of A / output per core
MT = RPC // 128  # 8 output m-tiles per core
KT = N // 128  # 64 contraction k-tiles

NF8 = 64  # k-tiles contracted in fp8 DoubleRow (must be even)
NF16 = KT - NF8  # k-tiles contracted in fp16
NP8 = NF8 // 2  # DoubleRow pair count
ACH = 2  # fp8 A panel chunks per m-tile
SC8 = NF8 // ACH  # fp8 k-subtiles per A chunk
PPC = SC8 // 2  # pairs per A chunk
XCH = 8  # x fp8 chunk count
SX = NF8 // XCH  # fp8 k-subtiles per x chunk

_F32 = mybir.dt.float32
_F16 = mybir.dt.float16
_FP8 = mybir.dt.float8e4
_NP_FP8 = ml_dtypes.float8_e4m3  # IEEE-ish e4m3 (max 240) == TRN FP8_EXP4
_DR = mybir.MatmulPerfMode.DoubleRow

_compiled = None
_last_in_maps = None


def _build(repeats=1):
    nc = bacc.Bacc("TRN2", target_bir_lowering=False, debug=False, num_devices=NCORES)
    # a8[m, p, s*128+c] = B[m*128+c, s*128+p] as e4m3  (s = k-subtile)
    a8 = nc.dram_tensor("a8", [MT, 128, NF8 * 128], _FP8, kind="ExternalInput").ap()
    # x8[p, s*512+d] = x[s*128+p, d] as e4m3
    x8 = nc.dram_tensor("x8", [128, NF8 * D], _FP8, kind="ExternalInput").ap()
    if NF16:
        a16 = nc.dram_tensor(
            "a16", [MT, 128, NF16 * 128], _F16, kind="ExternalInput"
        ).ap()
        x16 = nc.dram_tensor("x16", [128, NF16 * D], _F16, kind="ExternalInput").ap()
    # mean row 0.5*colsum(x) broadcast across partitions
    mb = nc.dram_tensor("mb", [128, D], _F32, kind="ExternalInput").ap()
    out = nc.dram_tensor("out", [RPC, D], _F32, kind="ExternalOutput").ap()

    with tile.TileContext(nc) as tc:
        with (
            tc.tile_pool(name="xp", bufs=1) as xp,
            tc.tile_pool(name="apool", bufs=3 * ACH) as apool,
            tc.tile_pool(name="a16pool", bufs=3) as a16pool,
            tc.tile_pool(name="mp", bufs=1) as mp,
            tc.tile_pool(name="op", bufs=4) as op,
            tc.tile_pool(name="ps", bufs=4, space="PSUM") as ps,
        ):
            for _rep in range(repeats):
                m_t = mp.tile([128, D], _F32, tag="m")
                nc.sync.dma_start(out=m_t[:], in_=mb[:, :])

                # x chunks stay resident for the whole rep (32KB/partition).
                x_tiles = [None] * XCH
                x16_tile = [None]

                def load_x(c):
                    xt = xp.tile([128, SX, D], _FP8, tag=f"x{c}")
                    nc.sync.dma_start(
                        out=xt[:], in_=x8[:, c * SX * D : (c + 1) * SX * D]
                    )
                    x_tiles[c] = xt

                def load_x16():
                    xt = xp.tile([128, NF16, D], _F16, tag="x16")
                    nc.sync.dma_start(out=xt[:], in_=x16[:, :])
                    x16_tile[0] = xt

                def load_a(m):
                    ts = []
                    for c in range(ACH):
                        a_t = apool.tile([128, SC8, 128], _FP8, tag="a8")
                        nc.sync.dma_start(
                            out=a_t[:],
                            in_=a8[m, :, c * SC8 * 128 : (c + 1) * SC8 * 128],
                        )
                        ts.append(a_t)
                    if NF16:
                        a_t = a16pool.tile([128, NF16, 128], _F16, tag="a16")
                        nc.sync.dma_start(out=a_t[:], in_=a16[m, :, :])
                        ts.append(a_t)
                    return ts

                # Issue order shapes DMA arrival order (HWDGE drains FIFO):
                # x0, A(m0), x1, A(m1), x2..x7, then A(m2..) in the loop.
                load_x(0)
                a_pending = {0: load_a(0)}
                load_x(1)
                a_pending[1] = load_a(1)
                for c in range(2, XCH):
                    load_x(c)
                if NF16:
                    load_x16()

                for m in range(MT):
                    a_tiles = a_pending.pop(m)
                    psum = ps.tile([128, D], _F32)
                    for t in range(NP8):
                        lhsT = a_tiles[t // PPC][
                            :, (t % PPC) * 2 : (t % PPC) * 2 + 2, :
                        ]
                        s = 2 * t
                        rhs = x_tiles[s // SX][:, (s % SX) : (s % SX) + 2, :]
                        nc.tensor.matmul(
                            psum[:],
                            lhsT,
                            rhs,
                            start=(t == 0),
                            stop=(t == NP8 - 1 and NF16 == 0),
                            perf_mode=_DR,
                        )
                    for j in range(NF16):
                        nc.tensor.matmul(
                            psum[:],
                            a_tiles[ACH][:, j : j + 1, :],
                            x16_tile[0][:, j : j + 1, :],
                            start=False,
                            stop=(j == NF16 - 1),
                        )
                    if m + 2 < MT:
                        a_pending[m + 2] = load_a(m + 2)
                    o_t = op.tile([128, D], _F32)
                    nc.vector.tensor_add(o_t[:], psum[:], m_t[:])
                    nc.sync.dma_start(
                        out=out[m * 128 : (m + 1) * 128, :], in_=o_t[:]
                    )

    nc.compile()
    return nc


def _get_compiled():
    global _compiled
    if _compiled is None:
        _compiled = _build()
    return _compiled


def _host_prep(input, A, W):
    """Quantize + rearrange full inputs into per-core in_maps."""
    x = input * W[None, :]  # fold diag(W) into the features
    # x8[p, s*512+d] = x[s*128+p, d]
    xr = x.reshape(KT, 128, D).transpose(1, 0, 2).reshape(128, KT * D)
    x8 = np.ascontiguousarray(xr[:, : NF8 * D]).astype(_NP_FP8)
    x16 = np.ascontiguousarray(xr[:, NF8 * D :]).astype(np.float16)

    mean_row = (0.5 * x.sum(axis=0, dtype=np.float64)).astype(np.float32)
    mb = np.ascontiguousarray(np.broadcast_to(mean_row[None, :], (128, D)))

    in_maps = []
    for i in range(NCORES):
        b_shard = A[i * RPC : (i + 1) * RPC] - np.float32(0.5)
        # atm[m, p, s*128+c] = b_shard[m*128+c, s*128+p]
        atm = (
            b_shard.reshape(MT, 128, KT, 128)
            .transpose(0, 3, 2, 1)
            .reshape(MT, 128, KT * 128)
        )
        a8 = np.ascontiguousarray(atm[:, :, : NF8 * 128]).astype(_NP_FP8)
        m = {"a8": a8, "x8": x8, "mb": mb}
        if NF16:
            m["a16"] = np.ascontiguousarray(atm[:, :, NF8 * 128 :]).astype(np.float16)
            m["x16"] = x16
        in_maps.append(m)
    return in_maps


def kernel(input, A, W):
    input = np.ascontiguousarray(np.asarray(input, dtype=np.float32))
    A = np.ascontiguousarray(np.asarray(A, dtype=np.float32))
    W = np.ascontiguousarray(np.asarray(W, dtype=np.float32))

    nc = _get_compiled()
    in_maps = _host_prep(input, A, W)

    global _last_in_maps
    _last_in_maps = in_maps

    res = run_bass_kernel_spmd(nc, in_maps, list(range(NCORES)))
    return np.concatenate(
        [np.asarray(res.results[i]["out"], dtype=np.float32) for i in range(NCORES)],
        axis=0,
    )
